# revision 18
# baseline (speedup 1.0000x reference)
"""Multi-head attention (B=512,S=64,D=1024,H=16) on 8 trn2 NeuronCores.

Strategy: pure data-parallel over the batch dim — each core gets 64 batches
(4096 tokens) and runs the full fused MHA layer locally; no collectives.

Per-core dataflow (token chunks of 512 = 8 batches):
  x [tok,1024] --PE transpose--> xT [1024,tok] (feature-major, bf16)
  qT = Wq.T @ xT, kT = Wk.T @ xT     (feature-major)
  v  = x @ Wv                        (token-major, interleaved with ones col)
  scoresT[k,q] = (kT slice).T @ (qT slice)   per (batch,head), quadrant packed
  expS = exp(scoresT/32)             (no max-subtract: logits are ~N(0,0.1))
  ctx[q,:]|sumexp[q] = expS.T @ [v|1]        -> normalize with per-partition recip
  ctxT via PE transpose; out = gelu(ctx @ Wo) accumulated token-major -> DRAM

All matmuls bf16 inputs with fp32 PSUM accumulation (rel err ~4e-3).

The emission order software-pipelines chunks: chunk ch's dense QKV projection
matmuls are interleaved with chunk ch-1's sparse attention matmuls so the
TensorE instruction stream never has a long low-duty stretch (keeps the PE
HAM clock-gate at 8/8 and hides the attention's ACT-exp latency).

PSUM packing rule (hardware): two concurrent matmuls may share a PSUM bank
only if they use the same array row-strip (same operand base partition) or
a strict diagonal (row,col) placement; different row-strips draining into
one bank is fatal. Scores use one PSUM tile per head-parity for this reason.
"""

import sys

sys.path.insert(0, "/opt/trn_rl_repo")

import numpy as np

import concourse.bass as bass
import concourse.tile as tile
from concourse import mybir
from concourse.bass_utils import run_bass_kernel_spmd
from concourse.masks import make_identity

F32 = mybir.dt.float32
BF = mybir.dt.bfloat16

B, S, D, H = 512, 64, 1024, 16
DH = D // H  # 64
NCORES = 8
BL = B // NCORES  # 64 batches per core
NTOK = BL * S  # 4096 tokens per core
CHUNK = 512  # tokens per pipeline chunk (8 batches)
NCH = NTOK // CHUNK  # 8
TT = CHUNK // 128  # 4 token-tiles per chunk
KT = D // 128  # 8 d-tiles
SCALE = 1.0 / np.sqrt(np.float32(D))  # 1/32


def _split_multiwait(nc, limit=1):
    """walrus can emit at most one sync-wait per instruction; TileContext's
    tail drain carries one wait per touched processor. Hoist extras onto
    chained NOPs."""
    f = nc.m.functions[0]
    for blk in f.blocks:
        new_insts = []
        for inst in blk.instructions:
            si = inst.sync_info
            if si is not None and len(si.on_wait) > limit:
                extra = si.on_wait[:-limit]
                keep = si.on_wait[-limit:]
                for i, w in enumerate(extra):
                    nop = mybir.InstNoOp(
                        name=f"{inst.name}-waitsplit{i}",
                        sync_info=mybir.SyncInfo(on_wait=[w], on_update=[]),
                        bass_nofuse=True,
                        ins=[],
                        outs=[],
                    )
                    nop.engine = inst.engine
                    new_insts.append(nop)
                si.on_wait[:] = keep
            new_insts.append(inst)
        blk.instructions[:] = new_insts


def _interleave(a, b):
    """Merge two unit lists round-robin, proportionally to their lengths."""
    out = []
    ia = ib = 0
    la, lb = len(a), len(b)
    while ia < la or ib < lb:
        if ib >= lb or (ia < la and ia * lb <= ib * la):
            out.append(a[ia])
            ia += 1
        else:
            out.append(b[ib])
            ib += 1
    return out


def build(split_waits=True):
    nc = bass.Bass("TRN2", debug=False, num_devices=NCORES)

    x_d = nc.declare_dram_parameter("x", [NTOK, D], F32, isOutput=False)
    w_d = {}
    b_d = {}
    for nm in ("wq", "wk", "wv", "wo"):
        w_d[nm] = nc.declare_dram_parameter(f"{nm}_w", [D, D], F32, isOutput=False)
        b_d[nm] = nc.declare_dram_parameter(f"{nm}_b", [D], F32, isOutput=False)
    out_d = nc.declare_dram_parameter("out", [NTOK, D], F32, isOutput=True)

    with tile.TileContext(nc) as tc:
        with (
            tc.tile_pool(name="weights", bufs=1) as wpool,
            tc.tile_pool(name="consts", bufs=1) as cpool,
            tc.tile_pool(name="wload", bufs=2) as ldpool,
            tc.tile_pool(name="xin", bufs=2) as xpool,
            tc.tile_pool(name="feat", bufs=2) as fpool,
            tc.tile_pool(name="attn", bufs=4) as apool,
            tc.tile_pool(name="outb", bufs=2) as opool,
            tc.tile_pool(name="psum", bufs=2, space="PSUM") as ppool,
        ):
            wt = {nm: [None] * KT for nm in ("wq", "wk", "wv", "wo")}
            biases = {}
            consts = {}

            def pe_transpose(src, dst):
                ps = ppool.tile([128, 128], BF, tag="tp", bufs=1, name="ps_tp")
                nc.tensor.transpose(ps, src, consts["identity"])
                nc.vector.tensor_copy(out=dst, in_=ps)

            def unit_load_weight(nm, k):
                def f():
                    wf = ldpool.tile([128, D], F32, tag="wload", name="wf")
                    nc.sync.dma_start(
                        out=wf[:], in_=w_d[nm][k * 128 : (k + 1) * 128, :]
                    )
                    wb = wpool.tile([128, D], BF, tag=f"w_{nm}_{k}", name=f"w{nm}{k}")
                    nc.vector.tensor_copy(out=wb[:], in_=wf[:])
                    wt[nm][k] = wb

                return f

            def unit_biases():
                def f():
                    # per-partition (feature-major) bias layout for q/k evac
                    for nm in ("wq", "wk"):
                        bt = cpool.tile([128, KT], F32, tag=f"{nm}_pb", name=f"{nm}_pb")
                        nc.sync.dma_start(
                            out=bt[:], in_=b_d[nm][:].rearrange("(m p) -> p m", p=128)
                        )
                        biases[nm] = bt
                    # broadcast-to-all-partitions bias tiles for v/o
                    ones_col = cpool.tile([1, 128], BF, tag="ones_col", name="ones_col")
                    nc.gpsimd.memset(ones_col[:], 1.0)
                    for nm in ("wv", "wo"):
                        row = ldpool.tile([1, D], F32, tag="wload", name="row")
                        nc.sync.dma_start(out=row[:], in_=b_d[nm][:].unsqueeze(0))
                        row_bf = ldpool.tile([1, D], BF, tag="rowbf", name="row_bf")
                        nc.vector.tensor_copy(out=row_bf[:], in_=row[:])
                        bc = cpool.tile([128, D], F32, tag=f"{nm}_bc", name=f"{nm}_bc")
                        for n in range(2):
                            psb = ppool.tile([128, 512], F32, tag="proj", bufs=3, name="psb")
                            nc.tensor.matmul(
                                psb[:],
                                lhsT=ones_col[:],
                                rhs=row_bf[:, n * 512 : (n + 1) * 512],
                                start=True,
                                stop=True,
                            )
                            nc.vector.tensor_copy(
                                out=bc[:, n * 512 : (n + 1) * 512], in_=psb[:]
                            )
                        biases[nm] = bc

                return f

            live = {}  # per-chunk tiles handed from stage A to stage B

            def stage_a_units(ch):
                """X load + transpose, then QKV projections for chunk ch."""
                tok0 = ch * CHUNK
                st = live.setdefault(ch, {})

                def u_x(t):
                    def f():
                        if "xT" not in st:
                            st["xT"] = [
                                fpool.tile([128, CHUNK], BF, tag=f"xT{k}", name=f"xT{k}")
                                for k in range(KT)
                            ]
                        xf = xpool.tile([128, D], F32, tag="xf32", name="xf")
                        nc.sync.dma_start(
                            out=xf[:], in_=x_d[tok0 + t * 128 : tok0 + (t + 1) * 128, :]
                        )
                        xb = xpool.tile([128, D], BF, tag="xbf", name="xb")
                        nc.vector.tensor_copy(out=xb[:], in_=xf[:])
                        for k in range(KT):
                            pe_transpose(
                                xb[:, k * 128 : (k + 1) * 128],
                                st["xT"][k][:, t * 128 : (t + 1) * 128],
                            )

                    return f

                def u_qk(which, m):
                    def f():
                        key = "qT" if which == "wq" else "kT"
                        if key not in st:
                            st[key] = [
                                fpool.tile([128, CHUNK], BF, tag=f"{key}{i}", name=f"{key}{i}")
                                for i in range(KT)
                            ]
                        ps = ppool.tile([128, CHUNK], F32, tag="proj", bufs=3, name="ps_qk")
                        for k in range(KT):
                            nc.tensor.matmul(
                                ps[:],
                                lhsT=wt[which][k][:, m * 128 : (m + 1) * 128],
                                rhs=st["xT"][k][:],
                                start=(k == 0),
                                stop=(k == KT - 1),
                            )
                        nc.scalar.activation(
                            out=st[key][m][:],
                            in_=ps[:],
                            func=mybir.ActivationFunctionType.Identity,
                            bias=biases[which][:, m : m + 1],
                        )

                    return f

                def u_v(t, n):
                    def f():
                        if "vaug" not in st:
                            st["vaug"] = [
                                apool.tile(
                                    [128, H * (DH + 1)], BF,
                                    tag=f"vaug{i}", name=f"vaug{i}", bufs=2,
                                )
                                for i in range(TT)
                            ]
                            for i in range(TT):
                                nc.gpsimd.memset(
                                    st["vaug"][i][:]
                                    .rearrange("p (h c) -> p h c", c=DH + 1)[:, :, DH : DH + 1],
                                    1.0,
                                )
                        ps = ppool.tile([128, CHUNK], F32, tag="proj", bufs=3, name="ps_v")
                        for k in range(KT):
                            nc.tensor.matmul(
                                ps[:],
                                lhsT=st["xT"][k][:, t * 128 : (t + 1) * 128],
                                rhs=wt["wv"][k][:, n * 512 : (n + 1) * 512],
                                start=(k == 0),
                                stop=(k == KT - 1),
                            )
                        nc.vector.tensor_tensor(
                            out=st["vaug"][t][:]
                            .rearrange("p (h c) -> p h c", c=DH + 1)[:, n * 8 : (n + 1) * 8, 0:DH],
                            in0=ps[:].rearrange("p (j c) -> p j c", c=DH),
                            in1=biases["wv"][:, n * 512 : (n + 1) * 512].rearrange(
                                "p (j c) -> p j c", c=DH
                            ),
                            op=mybir.AluOpType.add,
                        )

                    return f

                proj = []
                for m in range(KT):
                    proj.append(u_qk("wq", m))
                    proj.append(u_qk("wk", m))
                for t in range(TT):
                    for n in range(2):
                        proj.append(u_v(t, n))
                return {
                    "x": [u_x(t) for t in range(TT)],
                    "q": [u_qk("wq", m) for m in range(KT)],
                    "k": [u_qk("wk", m) for m in range(KT)],
                    "v": [u_v(t, n) for t in range(TT) for n in range(2)],
                    "proj": proj,
                }

            def attn_units(ch):
                """Attention for chunk ch, software-pipelined per batch-pair u:
                scores(t+1) is emitted before ctx(t) so the ACT exp latency is
                hidden behind the next head-pair's score matmuls; each u's
                ctx-transpose + output projection follows immediately."""
                st = live[ch]
                es_tiles = {}

                def u_scores(u, t):
                    def f():
                        qT, kT = st["qT"], st["kT"]
                        es = apool.tile([128, 128], BF, tag="expS", name="es")
                        es_tiles[(u, t)] = es
                        for hh in (0, 1):
                            hsl = slice(hh * 64, hh * 64 + 64)
                            ps_s = ppool.tile([128, 64], F32, tag="sc", bufs=2, name=f"ps_s{hh}")
                            for bpar in (0, 1):
                                toksl = slice(u * 128 + bpar * 64, u * 128 + bpar * 64 + 64)
                                nc.tensor.matmul(
                                    ps_s[bpar * 64 : bpar * 64 + 64, :],
                                    lhsT=kT[t][hsl, toksl],
                                    rhs=qT[t][hsl, toksl],
                                    start=True,
                                    stop=True,
                                )
                            nc.scalar.activation(
                                out=es[:, hsl],
                                in_=ps_s[:],
                                func=mybir.ActivationFunctionType.Exp,
                                scale=float(SCALE),
                            )

                    return f

                def u_ctx(u, t):
                    def f():
                        if "ctx" not in st:
                            st["ctx"] = [
                                apool.tile([128, D], BF, tag=f"ctx{i}", name=f"ctx{i}", bufs=2)
                                for i in range(TT)
                            ]
                        vaug, ctx = st["vaug"], st["ctx"]
                        es = es_tiles.pop((u, t))
                        ps_c = ppool.tile([128, 130], F32, tag="cx", bufs=2, name="ps_c")
                        for bpar in (0, 1):
                            bsl = slice(bpar * 64, bpar * 64 + 64)
                            for hh in (0, 1):
                                h = 2 * t + hh
                                nc.tensor.matmul(
                                    ps_c[bsl, hh * 65 : hh * 65 + 65],
                                    lhsT=es[bsl, hh * 64 : hh * 64 + 64],
                                    rhs=vaug[u][bsl, h * 65 : (h + 1) * 65],
                                    start=True,
                                    stop=True,
                                )
                        for hh in (0, 1):
                            h = 2 * t + hh
                            rc = apool.tile([128, 1], F32, tag="recip", name="rc")
                            nc.vector.reciprocal(
                                rc[:], ps_c[:, hh * 65 + DH : hh * 65 + DH + 1]
                            )
                            nc.vector.tensor_scalar(
                                out=ctx[u][:, h * DH : (h + 1) * DH],
                                in0=ps_c[:, hh * 65 : hh * 65 + DH],
                                scalar1=rc[:],
                                scalar2=None,
                                op0=mybir.AluOpType.mult,
                            )

                    return f

                units = []
                for u in range(TT):
                    units.append(u_scores(u, 0))
                    for t in range(KT - 1):
                        units.append(u_scores(u, t + 1))
                        units.append(u_ctx(u, t))
                    units.append(u_ctx(u, KT - 1))
                    units.extend(tail_units_u(ch, u))
                return units

            def tail_units_u(ch, u):
                """ctx transpose + output projection + gelu + store for one
                token-tile u of chunk ch (outproj(t=u) only needs ctxT(u))."""
                tok0 = ch * CHUNK
                st = live[ch]

                def u_ctxT(u):
                    def f():
                        if "cT" not in st:
                            st["cT"] = [
                                fpool.tile([128, CHUNK], BF, tag=f"cT{k}", name=f"cT{k}")
                                for k in range(KT)
                            ]
                        for k in range(KT):
                            pe_transpose(
                                st["ctx"][u][:, k * 128 : (k + 1) * 128],
                                st["cT"][k][:, u * 128 : (u + 1) * 128],
                            )

                    return f

                def u_out(t, n):
                    def f():
                        ps = ppool.tile([128, CHUNK], F32, tag="proj", bufs=3, name="ps_o")
                        for k in range(KT):
                            nc.tensor.matmul(
                                ps[:],
                                lhsT=st["cT"][k][:, t * 128 : (t + 1) * 128],
                                rhs=wt["wo"][k][:, n * 512 : (n + 1) * 512],
                                start=(k == 0),
                                stop=(k == KT - 1),
                            )
                        tmp = opool.tile([128, 512], F32, tag="obuf", name="tmp")
                        nc.vector.tensor_tensor(
                            out=tmp[:],
                            in0=ps[:],
                            in1=biases["wo"][:, n * 512 : (n + 1) * 512],
                            op=mybir.AluOpType.add,
                        )
                        og = opool.tile([128, 512], F32, tag="ogelu", name="og")
                        nc.scalar.activation(
                            out=og[:], in_=tmp[:], func=mybir.ActivationFunctionType.Gelu
                        )
                        nc.sync.dma_start(
                            out=out_d[
                                tok0 + t * 128 : tok0 + (t + 1) * 128,
                                n * 512 : (n + 1) * 512,
                            ],
                            in_=og[:],
                        )

                    return f

                return [u_ctxT(u), u_out(u, 0), u_out(u, 1)]

            # ---- emission ----
            # startup: wq + biases first (chunk 0's Q projection can start as
            # soon as they land), remaining weights interleaved with chunk 0.
            identity = cpool.tile([128, 128], BF, tag="ident", name="identity")
            make_identity(nc, identity[:])
            consts["identity"] = identity
            stages = [stage_a_units(ch) for ch in range(NCH)]
            # prologue: x(0), then chunk-0 projections interleaved with the
            # remaining weight loads and x(1)
            for t in range(TT):
                stages[0]["x"][t]()
            for k in range(KT):
                unit_load_weight("wq", k)()
            unit_biases()()
            for u in _interleave(stages[0]["q"], [unit_load_weight("wk", k) for k in range(KT)]):
                u()
            for u in _interleave(stages[0]["k"], [unit_load_weight("wv", k) for k in range(KT)]):
                u()
            for u in _interleave(
                _interleave(stages[0]["v"], stages[1]["x"]),
                [unit_load_weight("wo", k) for k in range(KT)],
            ):
                u()
            # steady state: block ch emits proj(ch) + x(ch+1) + attention(ch-1)
            for ch in range(1, NCH):
                dense = stages[ch]["proj"]
                if ch + 1 < NCH:
                    dense = _interleave(dense, stages[ch + 1]["x"])
                for u in _interleave(dense, attn_units(ch - 1)):
                    u()
                live.pop(ch - 1)
            for u in attn_units(NCH - 1):
                u()
            live.pop(NCH - 1)

    if split_waits:
        _split_multiwait(nc)
    return nc


_NC = None


def _get_nc():
    global _NC
    if _NC is None:
        _NC = build()
    return _NC


def _make_in_maps(inputs):
    x = np.ascontiguousarray(np.asarray(inputs["x"], dtype=np.float32))
    full = {
        nm: np.ascontiguousarray(np.asarray(inputs[nm], dtype=np.float32))
        for nm in ("wq_w", "wq_b", "wk_w", "wk_b", "wv_w", "wv_b", "wo_w", "wo_b")
    }
    in_maps = []
    for c in range(NCORES):
        m = {"x": np.ascontiguousarray(x[c * BL : (c + 1) * BL].reshape(NTOK, D))}
        m.update(full)
        in_maps.append(m)
    return in_maps


def kernel(**inputs):
    nc = _get_nc()
    res = run_bass_kernel_spmd(
        nc, _make_in_maps(inputs), core_ids=list(range(NCORES))
    ).results
    parts = [res[c]["out"].reshape(BL, 8, 8, D) for c in range(NCORES)]
    return np.concatenate(parts, axis=0)


def kernel_profiled(**inputs):
    """Like kernel() but requests an NTFF trace; returns (out, exec_time_ns, raw)."""
    nc = _get_nc()
    r = run_bass_kernel_spmd(
        nc, _make_in_maps(inputs), core_ids=list(range(NCORES)), trace=True
    )
    parts = [r.results[c]["out"].reshape(BL, 8, 8, D) for c in range(NCORES)]
    return np.concatenate(parts, axis=0), r.exec_time_ns, r


# revision 19
# speedup vs baseline: 1.0560x; 1.0560x over previous
"""Multi-head attention (B=512,S=64,D=1024,H=16) on 8 trn2 NeuronCores.

Strategy: pure data-parallel over the batch dim — each core gets 64 batches
(4096 tokens) and runs the full fused MHA layer locally; no collectives.

Per-core dataflow (token chunks of 512 = 8 batches):
  x [tok,1024] --PE transpose--> xT [1024,tok] (feature-major, bf16)
  qT = Wq.T @ xT, kT = Wk.T @ xT     (feature-major)
  v  = x @ Wv                        (token-major, interleaved with ones col)
  scoresT[k,q] = (kT slice).T @ (qT slice)   per (batch,head), quadrant packed
  expS = exp(scoresT/32)             (no max-subtract: logits are ~N(0,0.1))
  ctx[q,:]|sumexp[q] = expS.T @ [v|1]        -> normalize with per-partition recip
  ctxT via PE transpose; out = gelu(ctx @ Wo) accumulated token-major -> DRAM

All matmuls bf16 inputs with fp32 PSUM accumulation (rel err ~4e-3).

The emission order software-pipelines chunks: chunk ch's dense QKV projection
matmuls are interleaved with chunk ch-1's sparse attention matmuls so the
TensorE instruction stream never has a long low-duty stretch (keeps the PE
HAM clock-gate at 8/8 and hides the attention's ACT-exp latency).

PSUM packing rule (hardware): two concurrent matmuls may share a PSUM bank
only if they use the same array row-strip (same operand base partition) or
a strict diagonal (row,col) placement; different row-strips draining into
one bank is fatal. Scores use one PSUM tile per head-parity for this reason.
"""

import sys

sys.path.insert(0, "/opt/trn_rl_repo")

import numpy as np

import concourse.bass as bass
import concourse.tile as tile
from concourse import mybir
from concourse.bass_utils import run_bass_kernel_spmd
from concourse.masks import make_identity

F32 = mybir.dt.float32
BF = mybir.dt.bfloat16

B, S, D, H = 512, 64, 1024, 16
DH = D // H  # 64
NCORES = 8
BL = B // NCORES  # 64 batches per core
NTOK = BL * S  # 4096 tokens per core
CHUNK = 512  # tokens per pipeline chunk (8 batches)
NCH = NTOK // CHUNK  # 8
TT = CHUNK // 128  # 4 token-tiles per chunk
KT = D // 128  # 8 d-tiles
SCALE = 1.0 / np.sqrt(np.float32(D))  # 1/32


def _split_multiwait(nc, limit=1):
    """walrus can emit at most one sync-wait per instruction; TileContext's
    tail drain carries one wait per touched processor. Hoist extras onto
    chained NOPs."""
    f = nc.m.functions[0]
    for blk in f.blocks:
        new_insts = []
        for inst in blk.instructions:
            si = inst.sync_info
            if si is not None and len(si.on_wait) > limit:
                extra = si.on_wait[:-limit]
                keep = si.on_wait[-limit:]
                for i, w in enumerate(extra):
                    nop = mybir.InstNoOp(
                        name=f"{inst.name}-waitsplit{i}",
                        sync_info=mybir.SyncInfo(on_wait=[w], on_update=[]),
                        bass_nofuse=True,
                        ins=[],
                        outs=[],
                    )
                    nop.engine = inst.engine
                    new_insts.append(nop)
                si.on_wait[:] = keep
            new_insts.append(inst)
        blk.instructions[:] = new_insts


def _interleave(a, b):
    """Merge two unit lists round-robin, proportionally to their lengths."""
    out = []
    ia = ib = 0
    la, lb = len(a), len(b)
    while ia < la or ib < lb:
        if ib >= lb or (ia < la and ia * lb <= ib * la):
            out.append(a[ia])
            ia += 1
        else:
            out.append(b[ib])
            ib += 1
    return out


def build(split_waits=True):
    nc = bass.Bass("TRN2", debug=False, num_devices=NCORES)

    x_d = nc.declare_dram_parameter("x", [NTOK, D], F32, isOutput=False)
    w_d = {}
    b_d = {}
    for nm in ("wq", "wk", "wv", "wo"):
        w_d[nm] = nc.declare_dram_parameter(f"{nm}_w", [D, D], F32, isOutput=False)
        b_d[nm] = nc.declare_dram_parameter(f"{nm}_b", [D], F32, isOutput=False)
    out_d = nc.declare_dram_parameter("out", [NTOK, D], F32, isOutput=True)

    with tile.TileContext(nc) as tc:
        with (
            tc.tile_pool(name="weights", bufs=1) as wpool,
            tc.tile_pool(name="consts", bufs=1) as cpool,
            tc.tile_pool(name="wload", bufs=2) as ldpool,
            tc.tile_pool(name="xin", bufs=2) as xpool,
            tc.tile_pool(name="feat", bufs=2) as fpool,
            tc.tile_pool(name="attn", bufs=4) as apool,
            tc.tile_pool(name="outb", bufs=2) as opool,
            tc.tile_pool(name="psum", bufs=2, space="PSUM") as ppool,
        ):
            wt = {nm: [None] * KT for nm in ("wq", "wk", "wv", "wo")}
            biases = {}
            consts = {}

            def pe_transpose(src, dst):
                ps = ppool.tile([128, 128], BF, tag="tp", bufs=1, name="ps_tp")
                nc.tensor.transpose(ps, src, consts["identity"])
                nc.vector.tensor_copy(out=dst, in_=ps)

            def unit_load_weight(nm, k):
                def f():
                    wf = ldpool.tile([128, D], F32, tag="wload", name="wf")
                    nc.sync.dma_start(
                        out=wf[:], in_=w_d[nm][k * 128 : (k + 1) * 128, :]
                    )
                    wb = wpool.tile([128, D], BF, tag=f"w_{nm}_{k}", name=f"w{nm}{k}")
                    nc.vector.tensor_copy(out=wb[:], in_=wf[:])
                    wt[nm][k] = wb

                return f

            def unit_biases():
                def f():
                    # per-partition (feature-major) bias layout for q/k evac
                    for nm in ("wq", "wk"):
                        bt = cpool.tile([128, KT], F32, tag=f"{nm}_pb", name=f"{nm}_pb")
                        nc.sync.dma_start(
                            out=bt[:], in_=b_d[nm][:].rearrange("(m p) -> p m", p=128)
                        )
                        biases[nm] = bt
                    # broadcast-to-all-partitions bias tiles for v/o
                    ones_col = cpool.tile([1, 128], BF, tag="ones_col", name="ones_col")
                    nc.gpsimd.memset(ones_col[:], 1.0)
                    for nm in ("wv", "wo"):
                        row = ldpool.tile([1, D], F32, tag="wload", name="row")
                        nc.sync.dma_start(out=row[:], in_=b_d[nm][:].unsqueeze(0))
                        row_bf = ldpool.tile([1, D], BF, tag="rowbf", name="row_bf")
                        nc.vector.tensor_copy(out=row_bf[:], in_=row[:])
                        bc = cpool.tile([128, D], F32, tag=f"{nm}_bc", name=f"{nm}_bc")
                        for n in range(2):
                            psb = ppool.tile([128, 512], F32, tag="proj", bufs=2, name="psb")
                            nc.tensor.matmul(
                                psb[:],
                                lhsT=ones_col[:],
                                rhs=row_bf[:, n * 512 : (n + 1) * 512],
                                start=True,
                                stop=True,
                            )
                            nc.vector.tensor_copy(
                                out=bc[:, n * 512 : (n + 1) * 512], in_=psb[:]
                            )
                        biases[nm] = bc

                return f

            live = {}  # per-chunk tiles handed from stage A to stage B

            def stage_a_units(ch):
                """X load + transpose, then QKV projections for chunk ch."""
                tok0 = ch * CHUNK
                st = live.setdefault(ch, {})

                def u_x(t):
                    def f():
                        if "xT" not in st:
                            st["xT"] = [
                                fpool.tile([128, CHUNK], BF, tag=f"xT{k}", name=f"xT{k}")
                                for k in range(KT)
                            ]
                        xf = xpool.tile([128, D], F32, tag="xf32", name="xf")
                        nc.sync.dma_start(
                            out=xf[:], in_=x_d[tok0 + t * 128 : tok0 + (t + 1) * 128, :]
                        )
                        xb = xpool.tile([128, D], BF, tag="xbf", name="xb")
                        nc.vector.tensor_copy(out=xb[:], in_=xf[:])
                        for k in range(KT):
                            pe_transpose(
                                xb[:, k * 128 : (k + 1) * 128],
                                st["xT"][k][:, t * 128 : (t + 1) * 128],
                            )

                    return f

                def u_qk(which, m):
                    def f():
                        key = "qT" if which == "wq" else "kT"
                        if key not in st:
                            st[key] = [
                                fpool.tile([128, CHUNK], BF, tag=f"{key}{i}", name=f"{key}{i}")
                                for i in range(KT)
                            ]
                        ps = ppool.tile([128, CHUNK], F32, tag="proj", bufs=2, name="ps_qk")
                        for k in range(KT):
                            nc.tensor.matmul(
                                ps[:],
                                lhsT=wt[which][k][:, m * 128 : (m + 1) * 128],
                                rhs=st["xT"][k][:],
                                start=(k == 0),
                                stop=(k == KT - 1),
                            )
                        nc.scalar.activation(
                            out=st[key][m][:],
                            in_=ps[:],
                            func=mybir.ActivationFunctionType.Identity,
                            bias=biases[which][:, m : m + 1],
                        )

                    return f

                def u_v(t, n):
                    def f():
                        if "vaug" not in st:
                            st["vaug"] = [
                                apool.tile(
                                    [128, H * (DH + 1)], BF,
                                    tag=f"vaug{i}", name=f"vaug{i}", bufs=2,
                                )
                                for i in range(TT)
                            ]
                            for i in range(TT):
                                nc.gpsimd.memset(
                                    st["vaug"][i][:]
                                    .rearrange("p (h c) -> p h c", c=DH + 1)[:, :, DH : DH + 1],
                                    1.0,
                                )
                        ps = ppool.tile([128, CHUNK], F32, tag="proj", bufs=2, name="ps_v")
                        for k in range(KT):
                            nc.tensor.matmul(
                                ps[:],
                                lhsT=st["xT"][k][:, t * 128 : (t + 1) * 128],
                                rhs=wt["wv"][k][:, n * 512 : (n + 1) * 512],
                                start=(k == 0),
                                stop=(k == KT - 1),
                            )
                        nc.vector.tensor_tensor(
                            out=st["vaug"][t][:]
                            .rearrange("p (h c) -> p h c", c=DH + 1)[:, n * 8 : (n + 1) * 8, 0:DH],
                            in0=ps[:].rearrange("p (j c) -> p j c", c=DH),
                            in1=biases["wv"][:, n * 512 : (n + 1) * 512].rearrange(
                                "p (j c) -> p j c", c=DH
                            ),
                            op=mybir.AluOpType.add,
                        )

                    return f

                proj = []
                for m in range(KT):
                    proj.append(u_qk("wq", m))
                    proj.append(u_qk("wk", m))
                for t in range(TT):
                    for n in range(2):
                        proj.append(u_v(t, n))
                return {
                    "x": [u_x(t) for t in range(TT)],
                    "q": [u_qk("wq", m) for m in range(KT)],
                    "k": [u_qk("wk", m) for m in range(KT)],
                    "v": [u_v(t, n) for t in range(TT) for n in range(2)],
                    "proj": proj,
                }

            def attn_units(ch):
                """Attention for chunk ch, software-pipelined per batch-pair u:
                scores(t+1) is emitted before ctx(t) so the ACT exp latency is
                hidden behind the next head-pair's score matmuls; each u's
                ctx-transpose + output projection follows immediately."""
                st = live[ch]
                es_tiles = {}

                def u_scores(u, t):
                    def f():
                        qT, kT = st["qT"], st["kT"]
                        es = apool.tile([128, 128], BF, tag="expS", name="es")
                        es_tiles[(u, t)] = es
                        for hh in (0, 1):
                            hsl = slice(hh * 64, hh * 64 + 64)
                            ps_s = ppool.tile([128, 64], F32, tag="sc", bufs=3, name=f"ps_s{hh}")
                            for bpar in (0, 1):
                                toksl = slice(u * 128 + bpar * 64, u * 128 + bpar * 64 + 64)
                                nc.tensor.matmul(
                                    ps_s[bpar * 64 : bpar * 64 + 64, :],
                                    lhsT=kT[t][hsl, toksl],
                                    rhs=qT[t][hsl, toksl],
                                    start=True,
                                    stop=True,
                                )
                            nc.scalar.activation(
                                out=es[:, hsl],
                                in_=ps_s[:],
                                func=mybir.ActivationFunctionType.Exp,
                                scale=float(SCALE),
                            )

                    return f

                def u_ctx(u, t):
                    def f():
                        if "ctx" not in st:
                            st["ctx"] = [
                                apool.tile([128, D], BF, tag=f"ctx{i}", name=f"ctx{i}", bufs=2)
                                for i in range(TT)
                            ]
                        vaug, ctx = st["vaug"], st["ctx"]
                        es = es_tiles.pop((u, t))
                        ps_c = ppool.tile([128, 130], F32, tag="cx", bufs=2, name="ps_c")
                        for bpar in (0, 1):
                            bsl = slice(bpar * 64, bpar * 64 + 64)
                            for hh in (0, 1):
                                h = 2 * t + hh
                                nc.tensor.matmul(
                                    ps_c[bsl, hh * 65 : hh * 65 + 65],
                                    lhsT=es[bsl, hh * 64 : hh * 64 + 64],
                                    rhs=vaug[u][bsl, h * 65 : (h + 1) * 65],
                                    start=True,
                                    stop=True,
                                )
                        for hh in (0, 1):
                            h = 2 * t + hh
                            rc = apool.tile([128, 1], F32, tag="recip", name="rc")
                            nc.vector.reciprocal(
                                rc[:], ps_c[:, hh * 65 + DH : hh * 65 + DH + 1]
                            )
                            nc.vector.tensor_scalar(
                                out=ctx[u][:, h * DH : (h + 1) * DH],
                                in0=ps_c[:, hh * 65 : hh * 65 + DH],
                                scalar1=rc[:],
                                scalar2=None,
                                op0=mybir.AluOpType.mult,
                            )

                    return f

                units = []
                for u in range(TT):
                    units.append(u_scores(u, 0))
                    for t in range(KT - 1):
                        units.append(u_scores(u, t + 1))
                        units.append(u_ctx(u, t))
                    units.append(u_ctx(u, KT - 1))
                    units.extend(tail_units_u(ch, u))
                return units

            def tail_units_u(ch, u):
                """ctx transpose + output projection + gelu + store for one
                token-tile u of chunk ch (outproj(t=u) only needs ctxT(u))."""
                tok0 = ch * CHUNK
                st = live[ch]

                def u_ctxT(u):
                    def f():
                        if "cT" not in st:
                            st["cT"] = [
                                fpool.tile([128, CHUNK], BF, tag=f"cT{k}", name=f"cT{k}")
                                for k in range(KT)
                            ]
                        for k in range(KT):
                            pe_transpose(
                                st["ctx"][u][:, k * 128 : (k + 1) * 128],
                                st["cT"][k][:, u * 128 : (u + 1) * 128],
                            )

                    return f

                def u_out(t, n):
                    def f():
                        ps = ppool.tile([128, CHUNK], F32, tag="proj", bufs=2, name="ps_o")
                        for k in range(KT):
                            nc.tensor.matmul(
                                ps[:],
                                lhsT=st["cT"][k][:, t * 128 : (t + 1) * 128],
                                rhs=wt["wo"][k][:, n * 512 : (n + 1) * 512],
                                start=(k == 0),
                                stop=(k == KT - 1),
                            )
                        tmp = opool.tile([128, 512], F32, tag="obuf", name="tmp")
                        nc.vector.tensor_tensor(
                            out=tmp[:],
                            in0=ps[:],
                            in1=biases["wo"][:, n * 512 : (n + 1) * 512],
                            op=mybir.AluOpType.add,
                        )
                        og = opool.tile([128, 512], F32, tag="ogelu", name="og")
                        nc.scalar.activation(
                            out=og[:], in_=tmp[:], func=mybir.ActivationFunctionType.Gelu
                        )
                        nc.sync.dma_start(
                            out=out_d[
                                tok0 + t * 128 : tok0 + (t + 1) * 128,
                                n * 512 : (n + 1) * 512,
                            ],
                            in_=og[:],
                        )

                    return f

                return [u_ctxT(u), u_out(u, 0), u_out(u, 1)]

            # ---- emission ----
            # startup: wq + biases first (chunk 0's Q projection can start as
            # soon as they land), remaining weights interleaved with chunk 0.
            identity = cpool.tile([128, 128], BF, tag="ident", name="identity")
            make_identity(nc, identity[:])
            consts["identity"] = identity
            stages = [stage_a_units(ch) for ch in range(NCH)]
            # prologue: x(0), then chunk-0 projections interleaved with the
            # remaining weight loads and x(1)
            for t in range(TT):
                stages[0]["x"][t]()
            for k in range(KT):
                unit_load_weight("wq", k)()
            unit_biases()()
            for u in _interleave(
                _interleave(stages[0]["q"], stages[1]["x"][:2]),
                [unit_load_weight("wk", k) for k in range(KT)],
            ):
                u()
            for u in _interleave(
                _interleave(stages[0]["k"], stages[1]["x"][2:]),
                [unit_load_weight("wv", k) for k in range(KT)],
            ):
                u()
            for u in _interleave(
                stages[0]["v"],
                [unit_load_weight("wo", k) for k in range(KT)],
            ):
                u()
            # steady state: block ch emits proj(ch) + x(ch+1) + attention(ch-1)
            for ch in range(1, NCH):
                dense = stages[ch]["proj"]
                if ch + 1 < NCH:
                    dense = _interleave(dense, stages[ch + 1]["x"])
                for u in _interleave(dense, attn_units(ch - 1)):
                    u()
                live.pop(ch - 1)
            for u in attn_units(NCH - 1):
                u()
            live.pop(NCH - 1)

    if split_waits:
        _split_multiwait(nc)
    return nc


_NC = None


def _get_nc():
    global _NC
    if _NC is None:
        _NC = build()
    return _NC


def _make_in_maps(inputs):
    x = np.ascontiguousarray(np.asarray(inputs["x"], dtype=np.float32))
    full = {
        nm: np.ascontiguousarray(np.asarray(inputs[nm], dtype=np.float32))
        for nm in ("wq_w", "wq_b", "wk_w", "wk_b", "wv_w", "wv_b", "wo_w", "wo_b")
    }
    in_maps = []
    for c in range(NCORES):
        m = {"x": np.ascontiguousarray(x[c * BL : (c + 1) * BL].reshape(NTOK, D))}
        m.update(full)
        in_maps.append(m)
    return in_maps


def kernel(**inputs):
    nc = _get_nc()
    res = run_bass_kernel_spmd(
        nc, _make_in_maps(inputs), core_ids=list(range(NCORES))
    ).results
    parts = [res[c]["out"].reshape(BL, 8, 8, D) for c in range(NCORES)]
    return np.concatenate(parts, axis=0)


def kernel_profiled(**inputs):
    """Like kernel() but requests an NTFF trace; returns (out, exec_time_ns, raw)."""
    nc = _get_nc()
    r = run_bass_kernel_spmd(
        nc, _make_in_maps(inputs), core_ids=list(range(NCORES)), trace=True
    )
    parts = [r.results[c]["out"].reshape(BL, 8, 8, D) for c in range(NCORES)]
    return np.concatenate(parts, axis=0), r.exec_time_ns, r
